# revision 18
# baseline (speedup 1.0000x reference)
"""Butterfly permuter kernel for Trainium2 (8 NeuronCores, SPMD data-parallel).

The reference applies 10 butterfly rotation stages along the feature axis
(dim=1024) of x [16384, 1024].  Stages 1-9 (spans 2..512) only mix within the
two 512-wide halves of the feature axis, so their composition is
blockdiag(A, B) with dense 512x512 blocks; stage 10 (span 1024) rotates
element j with element j+512 using per-column angles.  We therefore compute

    z0 = x[:, :512] @ A          (PE, bf16, fp32 PSUM accumulation)
    z1 = x[:, 512:] @ B
    y[:, :512] = c*z0 + s*z1     (DVE/GpSimd elementwise, c/s = cos/sin of
    y[:, 512:] = c*z1 - s*z0      angles[9], broadcast along tokens)

which halves the PE matmul work vs the dense 1024x1024 formulation (the PE
moving-operand stream at 1 elem/cycle was the bottleneck) and puts the last
stage on the otherwise-idle vector engines.  bf16 operands (rel-err budget
2e-2; this lands ~2e-3) with fp32 accumulation and fp32 stage-10 arithmetic.

Per core: x_shard [2048, 1024]
  - SWDGE-DMA x chunks with inline fp32->bf16 cast
  - PE-transpose each [128 tok, 128 dim] bf16 block (bf16 identity, bf16
    PSUM) to get X^T blocks, evacuate PSUM->SBUF on ScalarE
  - per 128-token subtile: 2 accumulation groups of 4 bf16 matmuls
    (z0 += XT_kb^T @ A_kb, z1 += XT_kb^T @ B_kb), N=512
  - stage 10: 4 DVE mults (PSUM x coeff -> SBUF temps) + 2 GpSimd add/sub
    (temps -> y tile, fp32)
  - HWDGE-DMA y out in chunks on the ACT ring

Inputs arrive full-size; sharding is across the token axis (2048 rows/core).
"""

import numpy as np

import concourse.bass as bass
import concourse.mybir as mybir
import concourse.tile as tile
from concourse import bacc
from concourse.bass_utils import run_bass_kernel_spmd

N_CORES = 8
DIM = 1024
NUM_STAGES = 10
N_TOKENS = 16384
TOK_PER_CORE = N_TOKENS // N_CORES  # 2048
SUB = 128  # tokens per subtile (partition dim)
KB = DIM // 128  # 8 contraction blocks
HALF = DIM // 2

F32 = mybir.dt.float32
BF16 = mybir.dt.bfloat16
NP_BF16 = mybir.dt.np(BF16)
MULT = mybir.AluOpType.mult
ADD = mybir.AluOpType.add
SUBTRACT = mybir.AluOpType.subtract


def _compose(angles: np.ndarray, stages) -> np.ndarray:
    y = np.eye(DIM, dtype=np.float64)
    a = np.asarray(angles, dtype=np.float64)
    for s in stages:
        span = 2 ** (s + 1)
        half = span // 2
        y = y.reshape(-1, DIM // span, span)
        left, right = y[..., :half], y[..., half:]
        th = a[s].reshape(1, DIM // span, half)
        c, sn = np.cos(th), np.sin(th)
        y = np.concatenate([c * left + sn * right, -sn * left + c * right], -1)
        y = y.reshape(-1, DIM)
    return y


def compose_transform(angles: np.ndarray) -> np.ndarray:
    """Full dense R (float32) with y = x @ R (kept for reference/tests)."""
    return np.ascontiguousarray(_compose(angles, range(NUM_STAGES)),
                                dtype=np.float32)


def build_bass(reps: int = 1):
    """reps>1 repeats the whole pipeline in one NEFF (for marginal timing)."""
    nc = bacc.Bacc(None, target_bir_lowering=False)
    x = nc.dram_tensor("x", [TOK_PER_CORE, DIM], F32, kind="ExternalInput")
    # rows 0-511: A = R_{1..9}[:512, :512]; rows 512-1023: B (bottom-right)
    wlow = nc.dram_tensor("wlow", [DIM, HALF], BF16, kind="ExternalInput")
    # stage-10 [cos | sin | cos | sin], replicated to 128 partitions on host
    cs2 = nc.dram_tensor("cs2", [128, 4 * HALF], F32, kind="ExternalInput")
    ident = nc.dram_tensor("ident", [128, 128], BF16, kind="ExternalInput")
    y = nc.dram_tensor("y", [TOK_PER_CORE, DIM], F32, kind="ExternalOutput")

    n_sub = TOK_PER_CORE // SUB  # 16 subtiles of 128 tokens
    total_sub = reps * n_sub

    in_chunks = [2, 2, 4, 4, 4]
    out_chunks = [4, 4, 4, 2, 2]
    assert sum(in_chunks) == n_sub and sum(out_chunks) == n_sub
    in_start = [sum(in_chunks[:i]) for i in range(len(in_chunks))]
    out_start = [sum(out_chunks[:i]) for i in range(len(out_chunks))]
    sub_to_in_chunk = {}
    for ci, (st, ln) in enumerate(zip(in_start, in_chunks)):
        for s in range(st, st + ln):
            sub_to_in_chunk[s] = ci
    sub_to_out_chunk = {}
    for ci, (st, ln) in enumerate(zip(out_start, out_chunks)):
        for s in range(st, st + ln):
            sub_to_out_chunk[s] = ci

    # x-chunk loads are emitted LOOK subtiles before their first consumer so
    # the SWDGE descriptor generation (Pool engine, strict FIFO) is never
    # stuck behind a full rep's worth of stage-10 adds at rep boundaries.
    LOOK = 6
    load_sched = []  # (emit_at_global_subtile, rep, ci), in emission order
    for rep in range(reps):
        for ci, st in enumerate(in_start):
            load_sched.append((max(0, rep * n_sub + st - LOOK), rep, ci))
    load_sched.sort(key=lambda t: t[0])

    with tile.TileContext(nc) as tc:
        with (
            tc.tile_pool(name="const", bufs=1) as const_pool,
            tc.tile_pool(name="xin", bufs=4) as xin_pool,
            tc.tile_pool(name="xt", bufs=5) as xt_pool,
            tc.tile_pool(name="tmp", bufs=4) as tmp_pool,
            tc.tile_pool(name="yout", bufs=3) as yout_pool,
            tc.tile_pool(name="pst", bufs=2, space="PSUM") as pst_pool,
            tc.tile_pool(name="psz", bufs=3, space="PSUM") as psz_pool,
        ):
            ident_sb = const_pool.tile([128, 128], BF16, name="ident_sb")
            nc.sync.dma_start(ident_sb[:], ident[:])

            x_tiles = {}  # (rep, ci) -> tile
            y_tiles = {}  # (rep, co) -> tile

            def load_chunk(rep, ci):
                st, ln = in_start[ci], in_chunks[ci]
                x_tile = xin_pool.tile([128, ln * DIM], BF16, name="x_chunk",
                                       tag="x_chunk",
                                       padded_shape=[128, 4 * DIM])
                r0 = st * SUB
                # SWDGE: fp32 DRAM -> bf16 SBUF cast in the SDMA datapath.
                nc.gpsimd.dma_start(
                    x_tile[:, : ln * DIM].rearrange("p (s c) -> p s c", c=DIM),
                    x[r0 : r0 + ln * SUB, :].rearrange("(s p) c -> p s c", p=128),
                )
                x_tiles[(rep, ci)] = x_tile

            # A/B blocks: [128, kb*512] with a_sb[p, kb*512+j] = A[kb*128+p, j]
            a_sb = const_pool.tile([128, 4 * HALF], BF16, name="a_sb")
            b_sb = const_pool.tile([128, 4 * HALF], BF16, name="b_sb")
            for dst, r0 in ((a_sb, 0), (b_sb, HALF)):
                nc.sync.dma_start(
                    dst[:].rearrange("p (kb c) -> p kb c", c=HALF),
                    wlow[r0 : r0 + HALF, :].rearrange("(kb p) c -> p kb c", p=128),
                )
            cs_sb = const_pool.tile([128, 4 * HALF], F32, name="cs_sb")
            nc.sync.dma_start(cs_sb[:], cs2[:])

            xts = {}  # global subtile -> xt tile

            def emit_transpose(g):
                rep, s = divmod(g, n_sub)
                ci = sub_to_in_chunk[s]
                xcol = (s - in_start[ci]) * DIM
                x_tile = x_tiles[(rep, ci)]
                # one bank: [128, 1024] bf16 = 2KB/partition
                ps_t = pst_pool.tile([128, DIM], BF16, name="ps_t", tag="ps_t")
                for kb in range(KB):
                    nc.tensor.transpose(
                        ps_t[:, kb * 128 : (kb + 1) * 128],
                        x_tile[:, xcol + kb * 128 : xcol + (kb + 1) * 128],
                        ident_sb,
                    )
                xt = xt_pool.tile([128, DIM], BF16, name="xt", tag="xt")
                nc.scalar.copy(xt[:], ps_t[:])
                xts[g] = xt

            def emit_subtile(g):
                rep, s = divmod(g, n_sub)
                co = sub_to_out_chunk[s]
                st, ln = out_start[co], out_chunks[co]
                if s == st:
                    y_tiles[(rep, co)] = yout_pool.tile(
                        [128, ln * DIM], F32, name="y_chunk", tag="y_chunk",
                        padded_shape=[128, 4 * DIM],
                    )
                y_tile = y_tiles[(rep, co)]
                ycol = (s - st) * DIM
                xt = xts[g]
                # zP[:, :512] = sum XT_kb^T @ A_kb ; zP[:, 512:] = .. @ B_kb
                zp = psz_pool.tile([128, DIM], F32, name="zp", tag="ps_z")
                # stage-10 products: one broadcast DVE op per z-half,
                # emitted right after that half's accumulation group so the
                # z0 multiply overlaps the z1 matmuls on the PE.
                # t_all = [c*z0 | s*z0 | c*z1 | s*z1]
                t_all = tmp_pool.tile([128, 4 * 512], F32, name="t_all",
                                      tag="tmp")
                for zi, w_sb in enumerate((a_sb, b_sb)):
                    for k in range(4):
                        kb = zi * 4 + k
                        nc.tensor.matmul(
                            zp[:, zi * 512 : (zi + 1) * 512],
                            xt[:, kb * 128 : (kb + 1) * 128],
                            w_sb[:, k * 512 : (k + 1) * 512],
                            start=(k == 0),
                            stop=(k == 3),
                        )
                    z_b = (zp[:, zi * 512 : (zi + 1) * 512]
                           .unsqueeze(1).to_broadcast((128, 2, 512)))
                    cs_b = cs_sb[:, zi * 1024 : (zi + 1) * 1024].rearrange(
                        "p (r c) -> p r c", c=512)
                    t_b = t_all[:, zi * 1024 : (zi + 1) * 1024].rearrange(
                        "p (r c) -> p r c", c=512)
                    nc.vector.tensor_tensor(t_b, z_b, cs_b, MULT)
                # y0 = c*z0 + s*z1 ; y1 = c*z1 - s*z0  (GpSimd)
                nc.gpsimd.tensor_tensor(
                    y_tile[:, ycol : ycol + 512],
                    t_all[:, 0:512], t_all[:, 1536:2048], ADD)
                nc.gpsimd.tensor_tensor(
                    y_tile[:, ycol + 512 : ycol + 1024],
                    t_all[:, 1024:1536], t_all[:, 512:1024], SUBTRACT)
                if s == st + ln - 1:
                    r0 = st * SUB
                    # y stores on the ACT HWDGE ring (x loads are SWDGE).
                    nc.scalar.dma_start(
                        y[r0 : r0 + ln * SUB, :].rearrange("(s p) c -> p s c", p=128),
                        y_tile[:, : ln * DIM].rearrange("p (s c) -> p s c", c=DIM),
                    )

            # Transposes run SKEW subtiles ahead of the matmuls so the PE
            # never waits on the ScalarE PSUM->SBUF evacuation of its own
            # transpose outputs.
            # The whole NEFF is one global stream of reps*16 subtiles, so
            # both the load lookahead and the transpose skew flow across rep
            # boundaries.
            SKEW = 2
            li = 0

            def maybe_loads(g):
                nonlocal li
                while li < len(load_sched) and load_sched[li][0] <= g:
                    _, rep, ci = load_sched[li]
                    load_chunk(rep, ci)
                    li += 1

            maybe_loads(0)
            for p in range(min(SKEW, total_sub)):
                emit_transpose(p)
            for g in range(total_sub):
                maybe_loads(g)
                nxt = g + SKEW
                if nxt < total_sub:
                    emit_transpose(nxt)
                emit_subtile(g)
    nc.compile()
    return nc


_NC_CACHE = None


def _get_nc():
    global _NC_CACHE
    if _NC_CACHE is None:
        _NC_CACHE = build_bass()
    return _NC_CACHE


def host_inputs(x: np.ndarray, angles: np.ndarray):
    """Per-core input maps (x fp32 shards; weights/coeffs precomputed)."""
    x = np.ascontiguousarray(np.asarray(x, dtype=np.float32))
    angles = np.asarray(angles)
    r9 = _compose(angles, range(NUM_STAGES - 1))
    wlow = np.concatenate([r9[:HALF, :HALF], r9[HALF:, HALF:]], axis=0)
    wlow = np.ascontiguousarray(wlow.astype(NP_BF16))
    c = np.cos(np.asarray(angles[NUM_STAGES - 1], dtype=np.float64))
    s = np.sin(np.asarray(angles[NUM_STAGES - 1], dtype=np.float64))
    cs = np.concatenate([c, s, c, s]).astype(np.float32)  # [c|s|c|s]
    cs2 = np.ascontiguousarray(np.broadcast_to(cs, (128, 4 * HALF)))
    ident = np.eye(128, dtype=np.float32).astype(NP_BF16)
    return [
        {
            "x": x[c_ * TOK_PER_CORE : (c_ + 1) * TOK_PER_CORE],
            "wlow": wlow,
            "cs2": cs2,
            "ident": ident,
        }
        for c_ in range(N_CORES)
    ]


def run(x: np.ndarray, angles: np.ndarray, trace: bool = False):
    """Run on 8 cores; returns (y_full, BassKernelResults)."""
    nc = _get_nc()
    in_maps = host_inputs(x, angles)
    res = run_bass_kernel_spmd(
        nc, in_maps, core_ids=list(range(N_CORES)), trace=trace
    )
    y = np.concatenate([res.results[c]["y"] for c in range(N_CORES)], axis=0)
    return y, res


def kernel(x: np.ndarray, angles: np.ndarray) -> np.ndarray:
    y, _ = run(x, angles, trace=False)
    return y
